# revision 23
# baseline (speedup 1.0000x reference)
"""Trainium2 Bass kernel for nn_AttentionLayer (B=16, S=2048, D=512, H=64).

Data-parallel over batch: 8 NeuronCores x 2 batch items each; no collectives.

Math (per batch item b):
  qT = (Wq^T x^T + bq)/sqrt(H);  kT = Wk^T x^T + bk      [64, S] each
  w = exp(qT^T kT)  (no rowmax pass: |scores| <= ~9)
  out[h] = sum_t cbar[t] * v[t, :] / S + bv, cbar[t] = sum_s w[s,t]/Z[s]
  (uses sum_t cbar[t] == S so bv needs no extra scaling)

Structure:
  - ALL x transposes on the PE (no DRAM bounce, no DMA transposes):
    SWDGE casts HBM f32 -> SBUF xn bf16, then per-half groups of 32 PE
    transposes (bf16 identity, bf16 PSUM) evacuated to xT bf16 tiles by
    ScalarE (batch 0, idle prologue) / VectorE (batch 1, mid-strips).
    PE transposes double as HAM warmup.
  - fused projection stationary [Wq/sqrt(H) | Wk] -> A = [qT;kT];
    B = [kT;qT] via one PE permutation matmul per chunk.
  - scores row-strips [128,2048] via row-packed matmul pairs
    (tile_position (0,0)/(64,0)) streaming two column chunks at once.
  - exp SPLIT between ScalarE and VectorE per row-strip:
      * ScalarE strips: activation Exp (f32 PSUM -> bf16 SBUF) with
        accum_out rowsum.
      * DVE strips: Schraudolph bit-trick (i16 = s*log2e*128 + 16250.9 ==
        bf16 bits of exp(s)) via tensor_scalar f32 PSUM -> i16 SBUF.
        The colsum matmul reads the i16 tile DIRECTLY via .bitcast(bf16)
        (PE rhs reads of bitcast APs are safe on HW - measured; DVE and
        ScalarE compute ops on bitcast APs crash).  Rowsum: one byte-copy
        (sync HWDGE queue, transpose-free so no mode-switch hazards) to a
        real bf16 tile + tensor_add fold 2048->1024 + one 1x reduce_sum.
        accum_out on DVE is avoided (its CACHE_REDUCE costs a full 1x
        pass, ~2.3us).  DVE strips sit in colsum groups 0-2 only so the
        last group never waits on the byte-copy chain.
      Per-row softmax normalization cancels the Schraudolph per-element
      bias exactly (each row lives entirely on one engine).
  - colsum in groups of 4 strips (batched reciprocal), col-packed
    (0,32c) accumulation into one psum bank.
  - epilogue via V tiles: vT = Wv^T xT (PE), PE-transposed to v blocks
    [128t, 64h]; out = sum_t cbT[t] v[t]  -- 64-col streams instead of
    the old 512-col x-based streams, and no g/gT round-trips.
  - epilogue split in two parts with batch-1 strips issued in between so
    PE/ScalarE stay fed across the batch boundary.

Known landmines (measured on HW):
  - bitcast APs on DVE/ScalarE tensor ops -> NRT_EXEC_UNIT_UNRECOVERABLE.
    PE matmul rhs and DMA APs are safe.
  - matmul output must be fp32 (bass asserts); transpose output must
    match the input dtype (bf16 transposes write bf16 PSUM).
  - SWDGE DRAM->DRAM cast DMA -> INTERNAL crash.
  - gpsimd tensor_scalar with accum_out fails to compile; accum_out
    needs op1 set; gpsimd cannot read PSUM or reduce along free axis.
  - interleaving SBUF->SBUF copies with DMA transposes on the Sync queue
    causes multi-us mode-switch drains (moot now: no DMA transposes).
"""

import os as _os

import numpy as np

B, S, D, H = 16, 2048, 512, 64
NCORES = 8
BPC = B // NCORES  # batches per core
P = 128
NT = S // P  # 16 row strips
ND = D // P  # 4 d tiles
NC4 = S // 512  # 4 column chunks of 512

# Row strips whose exp runs on VectorE (Schraudolph); rest on ScalarE.
if _os.environ.get("K_DVE_EXP"):
    DVE_TILES = tuple(int(t) for t in _os.environ["K_DVE_EXP"].split(","))
else:
    _per = tuple(
        int(t) for t in _os.environ.get("K_DVE_PER", "1,3,6,9,12").split(",")
    )
    DVE_TILES = tuple(b * 16 + i for b in range(BPC) for i in _per)

# Schraudolph constants for bf16-bits exp: i16 = s * A16 + B16
A16 = float(np.log2(np.e) * 128.0)
B16 = 16250.91

N_WARMUP_MM = int(_os.environ.get("K_WARM", "48"))


def build_nc():
    import concourse.bacc as bacc
    import concourse.mybir as mybir
    import concourse.tile as tile

    f32 = mybir.dt.float32
    bf16 = mybir.dt.bfloat16
    i16 = mybir.dt.int16
    Exp = mybir.ActivationFunctionType.Exp
    Identity = mybir.ActivationFunctionType.Identity
    Copy = mybir.ActivationFunctionType.Copy
    X = mybir.AxisListType.X
    add = mybir.AluOpType.add
    mult = mybir.AluOpType.mult

    nc = bacc.Bacc("TRN2", target_bir_lowering=False)

    x_ext = nc.declare_dram_parameter("inputs", [BPC, S, D], f32, isOutput=False)
    wq_ext = nc.declare_dram_parameter("Wq", [D, H], f32, isOutput=False)
    bq_ext = nc.declare_dram_parameter("bq", [H], f32, isOutput=False)
    wk_ext = nc.declare_dram_parameter("Wk", [D, H], f32, isOutput=False)
    bk_ext = nc.declare_dram_parameter("bk", [H], f32, isOutput=False)
    wv_ext = nc.declare_dram_parameter("Wv", [D, H], f32, isOutput=False)
    bv_ext = nc.declare_dram_parameter("bv", [H], f32, isOutput=False)
    ident_ext = nc.declare_dram_parameter("ident128", [P, P], f32, isOutput=False)
    perm_ext = nc.declare_dram_parameter("perm64", [P, P], f32, isOutput=False)
    out_ext = nc.declare_dram_parameter("out", [BPC, H], f32, isOutput=True)

    inv_sqrt_h = 1.0 / float(np.sqrt(H))

    with tile.TileContext(nc) as tc:
        with (
            tc.tile_pool(name="singles", bufs=1) as singles,
            tc.tile_pool(name="xn", bufs=8) as xn_pool,
            tc.tile_pool(name="xT", bufs=16) as xT_pool,
            tc.tile_pool(name="qkT", bufs=4) as qkT_pool,
            tc.tile_pool(name="w", bufs=9) as w_pool,
            tc.tile_pool(name="sc16", bufs=4) as sc_pool,
            tc.tile_pool(name="wb", bufs=3) as wb_pool,
            tc.tile_pool(name="vt", bufs=2) as vt_pool,
            tc.tile_pool(name="vsb", bufs=8) as vsb_pool,
            tc.tile_pool(name="zr", bufs=6) as zr_pool,
            tc.tile_pool(name="misc", bufs=4) as misc_pool,
            tc.tile_pool(name="mm", bufs=3, space="PSUM") as mm_pool,
            tc.tile_pool(name="col", bufs=1, space="PSUM") as col_pool,
            tc.tile_pool(name="epi", bufs=1, space="PSUM") as epi_pool,
        ):
            # ---- PE warmup first: a memset tile needs no DMA, so the
            # HAM clock-gate release clock starts at ~0.3us
            warm_sb = singles.tile([P, P], bf16)
            nc.vector.memset(warm_sb, 0.001)
            warm_ps = epi_pool.tile([P, 512], f32, tag="epi", name="warm")
            for wi in range(N_WARMUP_MM):
                nc.tensor.matmul(
                    warm_ps[:, 0:P], lhsT=warm_sb, rhs=warm_sb,
                    start=True, stop=True,
                )

            # ---- constants / weights prep (once); scalar HWDGE queue ----
            ident_sb = singles.tile([P, P], f32)
            nc.scalar.dma_start(out=ident_sb, in_=ident_ext[:, :])
            perm_f = singles.tile([P, P], f32)
            nc.scalar.dma_start(out=perm_f, in_=perm_ext[:, :])
            perm_sb = singles.tile([P, P], bf16)
            nc.vector.tensor_copy(out=perm_sb, in_=perm_f)
            ident_b = singles.tile([P, P], bf16)
            nc.vector.tensor_copy(out=ident_b, in_=ident_sb)

            wq_f = singles.tile([P, ND, H], f32)
            nc.scalar.dma_start(out=wq_f, in_=wq_ext.rearrange("(j p) h -> p j h", p=P))
            wk_f = singles.tile([P, ND, H], f32)
            nc.scalar.dma_start(out=wk_f, in_=wk_ext.rearrange("(j p) h -> p j h", p=P))
            wv_f = singles.tile([P, ND, H], f32)
            nc.scalar.dma_start(out=wv_f, in_=wv_ext.rearrange("(j p) h -> p j h", p=P))

            # Fused projection stationary: wA = [Wq/sqrt(H) | Wk] per d-chunk
            wA = singles.tile([P, ND, P], bf16)
            for j in range(ND):
                nc.vector.tensor_scalar(
                    out=wA[:, j, 0:H], in0=wq_f[:, j, :],
                    scalar1=inv_sqrt_h, scalar2=None, op0=mult,
                )
                nc.vector.tensor_copy(out=wA[:, j, H:P], in_=wk_f[:, j, :])
            wv_b = singles.tile([P, ND, H], bf16)
            for j in range(ND):
                nc.vector.tensor_copy(out=wv_b[:, j, :], in_=wv_f[:, j, :])

            # biasA: rows 0:64 = bq/sqrt(H), rows 64:128 = bk
            biasA = singles.tile([P, 1], f32)
            nc.scalar.dma_start(out=biasA[0:H, 0:1], in_=bq_ext[:, None])
            nc.scalar.dma_start(out=biasA[H:P, 0:1], in_=bk_ext[:, None])
            nc.vector.tensor_scalar(
                out=biasA[0:H, 0:1], in0=biasA[0:H, 0:1],
                scalar1=inv_sqrt_h, scalar2=None, op0=mult,
            )
            bv_sb = singles.tile([1, H], f32)
            nc.scalar.dma_start(out=bv_sb, in_=bv_ext[None, :])

            # Preload the exp ACT table while ScalarE is idle.
            tbl_dummy = singles.tile([P, 1], f32)
            nc.scalar.activation(out=tbl_dummy, in_=ident_sb[:, 0:1], func=Exp)

            # ---- per-batch prologue state ----
            xn_tiles = [[None] * 4 for _ in range(BPC)]  # [b][k] -> [P, 4, 512]
            qkTA = [None] * BPC
            qkTB = [None] * BPC
            v_sbs = {0: [None] * 4, 1: [None] * 4}  # [b][g] -> [P, 4, H] bf16

            def cast_quarters(b, ks):
                """cast HBM f32 -> SBUF xn bf16 (SWDGE queue)."""
                for k in ks:
                    xn_bk = xn_pool.tile([P, 4, 512], bf16, tag="xn",
                                         name=f"xn{b}_{k}")
                    xv = x_ext[b, 512 * k : 512 * (k + 1), :].rearrange(
                        "(t p) d -> p t d", p=P
                    )
                    nc.gpsimd.dma_start(out=xn_bk, in_=xv)
                    xn_tiles[b][k] = xn_bk

            def pe_transpose_j(b, h, j, evac):
                """xT for one (s-half, d-block): 8 PE transposes + evac."""
                tp = mm_pool.tile([P, 1024], bf16, tag="mm",
                                  name=f"tp{b}_{j}_{h}")
                for u in range(8):
                    k = 2 * h + u // 4
                    tt = u % 4
                    nc.tensor.transpose(
                        out=tp[:, u * P : (u + 1) * P],
                        in_=xn_tiles[b][k][:, tt, j * P : (j + 1) * P],
                        identity=ident_b,
                    )
                xT_t = xT_pool.tile([P, 1024], bf16, tag="xT",
                                    name=f"xT{b}_{j}_{h}")
                if evac == "scalar":
                    nc.scalar.activation(out=xT_t[:, 0:512], in_=tp[:, 0:512],
                                         func=Copy)
                    nc.scalar.activation(out=xT_t[:, 512:1024],
                                         in_=tp[:, 512:1024], func=Copy)
                else:
                    nc.vector.tensor_copy(out=xT_t, in_=tp)
                return xT_t

            def pe_transpose_half(b, h, evac):
                return {j: pe_transpose_j(b, h, j, evac) for j in range(ND)}

            def proj_chunks(b, cs, xTs, evac_engine):
                """projection chunks: all A matmuls first, then permute-B."""
                if qkTA[b] is None:
                    qkTA[b] = qkT_pool.tile([P, S], bf16, tag="qkT", name=f"qkTA{b}")
                    qkTB[b] = qkT_pool.tile([P, S], bf16, tag="qkT", name=f"qkTB{b}")
                A, Bt = qkTA[b], qkTB[b]
                for c in cs:
                    sl = slice(c * 512, (c + 1) * 512)
                    ssl = slice((c % 2) * 512, (c % 2) * 512 + 512)
                    pa = epi_pool.tile([P, 512], f32, tag="epi", name=f"pa{b}_{c}")
                    for j in range(ND):
                        nc.tensor.matmul(
                            pa, lhsT=wA[:, j, :], rhs=xTs[j][:, ssl],
                            start=(j == 0), stop=(j == ND - 1),
                        )
                    if evac_engine == "scalar":
                        nc.scalar.activation(
                            out=A[:, sl], in_=pa, func=Identity, bias=biasA[:, 0:1]
                        )
                    else:
                        nc.vector.tensor_scalar(
                            out=A[:, sl], in0=pa,
                            scalar1=biasA[:, 0:1], scalar2=None, op0=add,
                        )
                for c in cs:
                    sl = slice(c * 512, (c + 1) * 512)
                    pb = epi_pool.tile([P, 512], f32, tag="epi", name=f"pb{b}_{c}")
                    nc.tensor.matmul(pb, lhsT=perm_sb, rhs=A[:, sl],
                                     start=True, stop=True)
                    if evac_engine == "scalar":
                        nc.scalar.activation(out=Bt[:, sl], in_=pb, func=Copy)
                    else:
                        nc.vector.tensor_copy(out=Bt[:, sl], in_=pb)

            def proj_v_h(b, h, xTs):
                """v blocks [128t, 64h] for one s-half: vT = Wv^T xT then
                PE transposes to [t-part, h] groups."""
                vT_sb = vt_pool.tile([H, 1024], bf16, tag="vt",
                                     name=f"vT{b}_{h}")
                for half2 in range(2):
                    vsl = slice(half2 * 512, half2 * 512 + 512)
                    vp = epi_pool.tile([H, 512], f32, tag="epi",
                                       name=f"vp{b}_{h}_{half2}")
                    for j in range(ND):
                        nc.tensor.matmul(
                            vp, lhsT=wv_b[:, j, :], rhs=xTs[j][:, vsl],
                            start=(j == 0), stop=(j == ND - 1),
                        )
                    nc.vector.tensor_copy(out=vT_sb[:, vsl], in_=vp)
                for gg in range(2):
                    g = 2 * h + gg
                    vps = epi_pool.tile([P, 4, H], bf16, tag="epi",
                                        name=f"vps{b}_{g}")
                    for u in range(4):
                        loc = 4 * gg + u
                        nc.tensor.transpose(
                            out=vps[:, u, :],
                            in_=vT_sb[0:H, loc * P : (loc + 1) * P],
                            identity=ident_b[0:H, 0:H],
                        )
                    v_g = vsb_pool.tile([P, 4, H], bf16, tag="v",
                                        name=f"v{b}_{g}")
                    nc.vector.tensor_copy(out=v_g, in_=vps)
                    v_sbs[b][g] = v_g

            # Per-strip storage
            w_store = {0: [None] * NT, 1: [None] * NT}
            w_rhs = {0: [None] * NT, 1: [None] * NT}

            def alloc_strip(b, i):
                if (16 * b + i) in DVE_TILES:
                    sct = sc_pool.tile([P, S], i16, tag="sc", name=f"sc{b}_{i}")
                    w_store[b][i] = sct
                    w_rhs[b][i] = sct.bitcast(bf16)
                else:
                    wt = w_pool.tile([P, S], bf16, tag="w", name=f"w{b}_{i}")
                    w_store[b][i] = wt
                    w_rhs[b][i] = wt

            # ---- scores + exp for one row strip ----
            def scores_tile(b, i, z_all, halves=(0, 1)):
                A, Bt = qkTA[b], qkTB[b]
                qsl = slice(i * P, (i + 1) * P)
                on_dve = (16 * b + i) in DVE_TILES
                wt = w_store[b][i]
                for half in halves:
                    c0, c1 = 2 * half, 2 * half + 1
                    ps = mm_pool.tile([P, 1024], f32, tag="mm",
                                      name=f"ps{b}_{i}_{half}")
                    nc.tensor.matmul(
                        ps[:, 0:512], lhsT=A[0:H, qsl],
                        rhs=Bt[0:H, c0 * 512 : (c0 + 1) * 512],
                        start=True, stop=True, tile_position=(0, 0),
                    )
                    nc.tensor.matmul(
                        ps[:, 512:1024], lhsT=Bt[H:P, qsl],
                        rhs=A[H:P, c1 * 512 : (c1 + 1) * 512],
                        start=True, stop=True, tile_position=(H, 0),
                    )
                    hs = slice(half * 1024, (half + 1) * 1024)
                    if on_dve:
                        nc.vector.tensor_scalar(
                            out=wt[:, hs], in0=ps,
                            scalar1=A16, scalar2=B16, op0=mult, op1=add,
                        )
                    else:
                        nc.scalar.activation(
                            out=wt[:, hs], in_=ps, func=Exp,
                            accum_out=z_all[:, i, half : half + 1],
                        )
                if on_dve and 1 in halves:
                    wbt = wb_pool.tile([P, S], bf16, tag="wb", name=f"wb{b}_{i}")
                    nc.sync.dma_start(out=wbt, in_=wt.bitcast(bf16))
                    nc.vector.tensor_add(
                        out=wbt[:, 0:1024], in0=wbt[:, 0:1024],
                        in1=wbt[:, 1024:2048],
                    )
                    nc.vector.reduce_sum(
                        out=z_all[:, i, 0:1], in_=wbt[:, 0:1024], axis=X
                    )

            def colsum_group(b, i0, z_all, rz_all, rzb_all, colbank, n=4):
                nc.vector.reduce_sum(
                    out=rz_all[:, i0 : i0 + n], in_=z_all[:, i0 : i0 + n, :], axis=X
                )
                nc.vector.reciprocal(
                    out=rz_all[:, i0 : i0 + n], in_=rz_all[:, i0 : i0 + n]
                )
                nc.vector.tensor_copy(
                    out=rzb_all[:, i0 : i0 + n], in_=rz_all[:, i0 : i0 + n]
                )
                for i in range(i0, i0 + n):
                    for c in range(NC4):
                        nc.tensor.matmul(
                            colbank[32 * c : 32 * c + 1, :],
                            lhsT=rzb_all[:, i : i + 1],
                            rhs=w_rhs[b][i][:, c * 512 : (c + 1) * 512],
                            start=(i == 0), stop=(i == NT - 1),
                            tile_position=(0, 32 * c),
                        )

            def epilogue_a(b, colbank):
                """cbar evac + transpose + extract (DVE/PE round trip)."""
                cbar_sb = misc_pool.tile([P, 512], f32, tag="cbar", name=f"cbar{b}")
                nc.vector.tensor_copy(out=cbar_sb, in_=colbank)
                cbT_ps = epi_pool.tile([P, 512], f32, tag="epi", name=f"cbT{b}")
                for f in range(4):
                    nc.tensor.transpose(
                        out=cbT_ps[:, f * P : (f + 1) * P],
                        in_=cbar_sb[:, f * P : (f + 1) * P],
                        identity=ident_sb,
                    )
                cbT_sb = misc_pool.tile([P, 4, 4], bf16, tag="cbT", name=f"cbT{b}")
                src = cbT_ps[:, :].rearrange("p (f c r) -> p c f r", f=4, c=4, r=32)
                nc.vector.tensor_copy(out=cbT_sb, in_=src[:, :, :, 0])
                return cbT_sb

            def epilogue_b(b, cbT_sb):
                fp = epi_pool.tile([1, H], f32, tag="epi", name=f"fp{b}")
                for t in range(NT):
                    nc.tensor.matmul(
                        fp, lhsT=cbT_sb[:, t // 4, t % 4 : t % 4 + 1],
                        rhs=v_sbs[b][t // 4][:, t % 4, :],
                        start=(t == 0), stop=(t == NT - 1),
                    )
                o_sb = misc_pool.tile([1, H], f32, tag="o", name=f"o{b}")
                nc.vector.scalar_tensor_tensor(
                    out=o_sb, in0=fp, scalar=1.0 / float(S), in1=bv_sb,
                    op0=mult, op1=add,
                )
                nc.sync.dma_start(out=out_ext[b : b + 1, :], in_=o_sb)

            # ================= schedule =================
            z_alls = {}
            rz_alls = {}
            rzb_alls = {}
            colbanks = {}
            for b in range(BPC):
                z_alls[b] = zr_pool.tile([P, NT, 2], f32, tag="z", name=f"z{b}")
                nc.vector.memset(z_alls[b], 0.0)
                rz_alls[b] = zr_pool.tile([P, NT], f32, tag="rz", name=f"rz{b}")
                rzb_alls[b] = zr_pool.tile([P, NT], bf16, tag="rzb", name=f"rzb{b}")

            def get_colbank(b):
                cb = col_pool.tile([P, 512], f32, tag="col", name=f"cb{b}")
                nc.vector.memset(cb, 0.0)
                colbanks[b] = cb

            # batch 0 prologue: casts up-front, PE transposes, scalar evacs
            cast_quarters(0, (0, 1))
            cast_quarters(0, (2, 3))
            xT00 = pe_transpose_half(0, 0, "scalar")
            proj_chunks(0, (0, 1), xT00, "vector")
            xT01 = pe_transpose_half(0, 1, "vector")

            get_colbank(0)
            za0 = z_alls[0]
            # only 3 strips need the h0/h1 split -- just enough to hide the
            # proj(2,3) latency; the rest run as full strips
            for i in range(3):
                alloc_strip(0, i)
                scores_tile(0, i, za0, halves=(0,))
            proj_chunks(0, (2, 3), xT01, "scalar")
            for i in range(3):
                scores_tile(0, i, za0, halves=(1,))
            for i in range(3, 6):
                alloc_strip(0, i)
                scores_tile(0, i, za0)
                if i == 5:
                    colsum_group(0, 0, za0, rz_alls[0], rzb_alls[0], colbanks[0])

            cast_quarters(1, (0, 1))
            cast_quarters(1, (2, 3))

            for i in range(6, 8):
                alloc_strip(0, i)
                scores_tile(0, i, za0)
            # interleave batch-1 PE transposes (per d-block) between strips
            # so they never block strip pair matmuls in the PE queue
            xT10 = {}
            xT11 = {}
            for i in range(8, 12):
                alloc_strip(0, i)
                scores_tile(0, i, za0)
                xT10[i - 8] = pe_transpose_j(1, 0, i - 8, "vector")
                if i == 9:
                    colsum_group(0, 4, za0, rz_alls[0], rzb_alls[0], colbanks[0])

            for i in range(12, NT):
                alloc_strip(0, i)
                scores_tile(0, i, za0)
                xT11[i - 12] = pe_transpose_j(1, 1, i - 12, "vector")
                if i == 13:
                    colsum_group(0, 8, za0, rz_alls[0], rzb_alls[0], colbanks[0])
                    proj_chunks(1, (0, 1), xT10, "vector")

            proj_chunks(1, (2, 3), xT11, "vector")

            colsum_group(0, 12, z_alls[0], rz_alls[0], rzb_alls[0],
                         colbanks[0], n=2)
            colsum_group(0, 14, z_alls[0], rz_alls[0], rzb_alls[0],
                         colbanks[0], n=2)
            cbT0 = epilogue_a(0, colbanks[0])

            # batch-1: all proj chunks are ready before its first strip, so
            # run FULL strips (the b0-style halves-split only exists to hide
            # proj latency) with v/epilogue pieces spread thinly between them
            get_colbank(1)
            za1 = z_alls[1]
            for i in range(NT):
                alloc_strip(1, i)
                scores_tile(1, i, za1)
                if i == 1:
                    proj_v_h(1, 0, xT10)
                if i == 3:
                    proj_v_h(0, 0, xT00)
                if i == 5:
                    proj_v_h(0, 1, xT01)
                    colsum_group(1, 0, za1, rz_alls[1], rzb_alls[1], colbanks[1])
                if i == 7:
                    epilogue_b(0, cbT0)
                if i == 9:
                    colsum_group(1, 4, za1, rz_alls[1], rzb_alls[1], colbanks[1])
                if i == 10:
                    proj_v_h(1, 1, xT11)
                if i == 13:
                    colsum_group(1, 8, za1, rz_alls[1], rzb_alls[1], colbanks[1])
                if i == 14:
                    colsum_group(1, 12, za1, rz_alls[1], rzb_alls[1],
                                 colbanks[1], n=2)
            colsum_group(1, 14, z_alls[1], rz_alls[1], rzb_alls[1],
                         colbanks[1], n=2)
            cbT1 = epilogue_a(1, colbanks[1])
            epilogue_b(1, cbT1)

    nc.finalize()
    return nc


_NC_CACHE = None


def _get_nc():
    global _NC_CACHE
    if _NC_CACHE is None:
        _NC_CACHE = build_nc()
    return _NC_CACHE


def run(inputs_map, trace=False, **spmd_kwargs):
    from concourse.bass_utils import run_bass_kernel_spmd

    x = np.ascontiguousarray(np.asarray(inputs_map["inputs"], dtype=np.float32))
    assert x.shape == (B, S, D), x.shape
    full = {
        "Wq": np.ascontiguousarray(np.asarray(inputs_map["Wq"], np.float32)),
        "bq": np.ascontiguousarray(np.asarray(inputs_map["bq"], np.float32)),
        "Wk": np.ascontiguousarray(np.asarray(inputs_map["Wk"], np.float32)),
        "bk": np.ascontiguousarray(np.asarray(inputs_map["bk"], np.float32)),
        "Wv": np.ascontiguousarray(np.asarray(inputs_map["Wv"], np.float32)),
        "bv": np.ascontiguousarray(np.asarray(inputs_map["bv"], np.float32)),
        "ident128": np.eye(P, dtype=np.float32),
        "perm64": np.roll(np.eye(P, dtype=np.float32), 64, axis=0),
    }
    in_maps = []
    for i in range(NCORES):
        m = {"inputs": np.ascontiguousarray(x[i * BPC : (i + 1) * BPC])}
        m.update(full)
        in_maps.append(m)
    nc = _get_nc()
    res = run_bass_kernel_spmd(
        nc, in_maps, core_ids=list(range(NCORES)), trace=trace, **spmd_kwargs
    )
    out = np.concatenate([np.asarray(res.results[i]["out"]) for i in range(NCORES)], 0)
    return out.astype(np.float32), res


def kernel(**inputs):
    out, _ = run(inputs, trace=False)
    return out


if __name__ == "__main__":
    rng = np.random.default_rng(0)
    ins = {
        "inputs": rng.standard_normal((B, S, D), dtype=np.float32),
        "Wq": rng.standard_normal((D, H), dtype=np.float32) / np.sqrt(D),
        "bq": np.zeros(H, np.float32),
        "Wk": rng.standard_normal((D, H), dtype=np.float32) / np.sqrt(D),
        "bk": np.zeros(H, np.float32),
        "Wv": rng.standard_normal((D, H), dtype=np.float32) / np.sqrt(D),
        "bv": np.zeros(H, np.float32),
    }
    out = kernel(**ins)
    print("out", out.shape, out[0, :4])


# revision 24
# speedup vs baseline: 1.0114x; 1.0114x over previous
"""Trainium2 Bass kernel for nn_AttentionLayer (B=16, S=2048, D=512, H=64).

Data-parallel over batch: 8 NeuronCores x 2 batch items each; no collectives.

Math (per batch item b):
  qT = (Wq^T x^T + bq)/sqrt(H);  kT = Wk^T x^T + bk      [64, S] each
  w = exp(qT^T kT)  (no rowmax pass: |scores| <= ~9)
  out[h] = sum_t cbar[t] * v[t, :] / S + bv, cbar[t] = sum_s w[s,t]/Z[s]
  (uses sum_t cbar[t] == S so bv needs no extra scaling)

Structure:
  - ALL x transposes on the PE (no DRAM bounce, no DMA transposes):
    SWDGE casts HBM f32 -> SBUF xn bf16, then per-half groups of 32 PE
    transposes (bf16 identity, bf16 PSUM) evacuated to xT bf16 tiles by
    ScalarE (batch 0, idle prologue) / VectorE (batch 1, mid-strips).
    PE transposes double as HAM warmup.
  - fused projection stationary [Wq/sqrt(H) | Wk] -> A = [qT;kT];
    B = [kT;qT] via one PE permutation matmul per chunk.
  - scores row-strips [128,2048] via row-packed matmul pairs
    (tile_position (0,0)/(64,0)) streaming two column chunks at once.
  - exp SPLIT between ScalarE and VectorE per row-strip:
      * ScalarE strips: activation Exp (f32 PSUM -> bf16 SBUF) with
        accum_out rowsum.
      * DVE strips: Schraudolph bit-trick (i16 = s*log2e*128 + 16250.9 ==
        bf16 bits of exp(s)) via tensor_scalar f32 PSUM -> i16 SBUF.
        The colsum matmul reads the i16 tile DIRECTLY via .bitcast(bf16)
        (PE rhs reads of bitcast APs are safe on HW - measured; DVE and
        ScalarE compute ops on bitcast APs crash).  Rowsum: one byte-copy
        (sync HWDGE queue, transpose-free so no mode-switch hazards) to a
        real bf16 tile + tensor_add fold 2048->1024 + one 1x reduce_sum.
        accum_out on DVE is avoided (its CACHE_REDUCE costs a full 1x
        pass, ~2.3us).  DVE strips sit in colsum groups 0-2 only so the
        last group never waits on the byte-copy chain.
      Per-row softmax normalization cancels the Schraudolph per-element
      bias exactly (each row lives entirely on one engine).
  - colsum in groups of 4 strips (batched reciprocal), col-packed
    (0,32c) accumulation into one psum bank.
  - epilogue via V tiles: vT = Wv^T xT (PE), PE-transposed to v blocks
    [128t, 64h]; out = sum_t cbT[t] v[t]  -- 64-col streams instead of
    the old 512-col x-based streams, and no g/gT round-trips.
  - epilogue split in two parts with batch-1 strips issued in between so
    PE/ScalarE stay fed across the batch boundary.

Known landmines (measured on HW):
  - bitcast APs on DVE/ScalarE tensor ops -> NRT_EXEC_UNIT_UNRECOVERABLE.
    PE matmul rhs and DMA APs are safe.
  - matmul output must be fp32 (bass asserts); transpose output must
    match the input dtype (bf16 transposes write bf16 PSUM).
  - SWDGE DRAM->DRAM cast DMA -> INTERNAL crash.
  - gpsimd tensor_scalar with accum_out fails to compile; accum_out
    needs op1 set; gpsimd cannot read PSUM or reduce along free axis.
  - interleaving SBUF->SBUF copies with DMA transposes on the Sync queue
    causes multi-us mode-switch drains (moot now: no DMA transposes).
"""

import os as _os

import numpy as np

B, S, D, H = 16, 2048, 512, 64
NCORES = 8
BPC = B // NCORES  # batches per core
P = 128
NT = S // P  # 16 row strips
ND = D // P  # 4 d tiles
NC4 = S // 512  # 4 column chunks of 512

# Row strips whose exp runs on VectorE (Schraudolph); rest on ScalarE.
if _os.environ.get("K_DVE_EXP"):
    DVE_TILES = tuple(int(t) for t in _os.environ["K_DVE_EXP"].split(","))
else:
    _per = tuple(
        int(t) for t in _os.environ.get("K_DVE_PER", "1,3,6,9,11").split(",")
    )
    DVE_TILES = tuple(b * 16 + i for b in range(BPC) for i in _per)

# Schraudolph constants for bf16-bits exp: i16 = s * A16 + B16
A16 = float(np.log2(np.e) * 128.0)
B16 = 16250.91

N_WARMUP_MM = int(_os.environ.get("K_WARM", "48"))


def build_nc():
    import concourse.bacc as bacc
    import concourse.mybir as mybir
    import concourse.tile as tile

    f32 = mybir.dt.float32
    bf16 = mybir.dt.bfloat16
    i16 = mybir.dt.int16
    Exp = mybir.ActivationFunctionType.Exp
    Identity = mybir.ActivationFunctionType.Identity
    Copy = mybir.ActivationFunctionType.Copy
    X = mybir.AxisListType.X
    add = mybir.AluOpType.add
    mult = mybir.AluOpType.mult

    nc = bacc.Bacc("TRN2", target_bir_lowering=False)

    x_ext = nc.declare_dram_parameter("inputs", [BPC, S, D], f32, isOutput=False)
    wq_ext = nc.declare_dram_parameter("Wq", [D, H], f32, isOutput=False)
    bq_ext = nc.declare_dram_parameter("bq", [H], f32, isOutput=False)
    wk_ext = nc.declare_dram_parameter("Wk", [D, H], f32, isOutput=False)
    bk_ext = nc.declare_dram_parameter("bk", [H], f32, isOutput=False)
    wv_ext = nc.declare_dram_parameter("Wv", [D, H], f32, isOutput=False)
    bv_ext = nc.declare_dram_parameter("bv", [H], f32, isOutput=False)
    ident_ext = nc.declare_dram_parameter("ident128", [P, P], f32, isOutput=False)
    perm_ext = nc.declare_dram_parameter("perm64", [P, P], f32, isOutput=False)
    out_ext = nc.declare_dram_parameter("out", [BPC, H], f32, isOutput=True)

    inv_sqrt_h = 1.0 / float(np.sqrt(H))

    with tile.TileContext(nc) as tc:
        with (
            tc.tile_pool(name="singles", bufs=1) as singles,
            tc.tile_pool(name="xn", bufs=8) as xn_pool,
            tc.tile_pool(name="xT", bufs=16) as xT_pool,
            tc.tile_pool(name="qkT", bufs=4) as qkT_pool,
            tc.tile_pool(name="w", bufs=9) as w_pool,
            tc.tile_pool(name="sc16", bufs=4) as sc_pool,
            tc.tile_pool(name="wb", bufs=3) as wb_pool,
            tc.tile_pool(name="vt", bufs=2) as vt_pool,
            tc.tile_pool(name="vsb", bufs=8) as vsb_pool,
            tc.tile_pool(name="zr", bufs=6) as zr_pool,
            tc.tile_pool(name="misc", bufs=4) as misc_pool,
            tc.tile_pool(name="mm", bufs=3, space="PSUM") as mm_pool,
            tc.tile_pool(name="col", bufs=1, space="PSUM") as col_pool,
            tc.tile_pool(name="epi", bufs=1, space="PSUM") as epi_pool,
        ):
            # ---- PE warmup first: a memset tile needs no DMA, so the
            # HAM clock-gate release clock starts at ~0.3us
            warm_sb = singles.tile([P, P], bf16)
            nc.vector.memset(warm_sb, 0.001)
            warm_ps = epi_pool.tile([P, 512], f32, tag="epi", name="warm")
            for wi in range(N_WARMUP_MM):
                nc.tensor.matmul(
                    warm_ps[:, 0:P], lhsT=warm_sb, rhs=warm_sb,
                    start=True, stop=True,
                )

            # ---- constants / weights prep (once); scalar HWDGE queue ----
            ident_sb = singles.tile([P, P], f32)
            nc.scalar.dma_start(out=ident_sb, in_=ident_ext[:, :])
            perm_f = singles.tile([P, P], f32)
            nc.scalar.dma_start(out=perm_f, in_=perm_ext[:, :])
            perm_sb = singles.tile([P, P], bf16)
            nc.vector.tensor_copy(out=perm_sb, in_=perm_f)
            ident_b = singles.tile([P, P], bf16)
            nc.vector.tensor_copy(out=ident_b, in_=ident_sb)

            wq_f = singles.tile([P, ND, H], f32)
            nc.scalar.dma_start(out=wq_f, in_=wq_ext.rearrange("(j p) h -> p j h", p=P))
            wk_f = singles.tile([P, ND, H], f32)
            nc.scalar.dma_start(out=wk_f, in_=wk_ext.rearrange("(j p) h -> p j h", p=P))
            wv_f = singles.tile([P, ND, H], f32)
            nc.scalar.dma_start(out=wv_f, in_=wv_ext.rearrange("(j p) h -> p j h", p=P))

            # Fused projection stationary: wA = [Wq/sqrt(H) | Wk] per d-chunk
            wA = singles.tile([P, ND, P], bf16)
            for j in range(ND):
                nc.vector.tensor_scalar(
                    out=wA[:, j, 0:H], in0=wq_f[:, j, :],
                    scalar1=inv_sqrt_h, scalar2=None, op0=mult,
                )
                nc.vector.tensor_copy(out=wA[:, j, H:P], in_=wk_f[:, j, :])
            wv_b = singles.tile([P, ND, H], bf16)
            for j in range(ND):
                nc.vector.tensor_copy(out=wv_b[:, j, :], in_=wv_f[:, j, :])

            # biasA: rows 0:64 = bq/sqrt(H), rows 64:128 = bk
            biasA = singles.tile([P, 1], f32)
            nc.scalar.dma_start(out=biasA[0:H, 0:1], in_=bq_ext[:, None])
            nc.scalar.dma_start(out=biasA[H:P, 0:1], in_=bk_ext[:, None])
            nc.vector.tensor_scalar(
                out=biasA[0:H, 0:1], in0=biasA[0:H, 0:1],
                scalar1=inv_sqrt_h, scalar2=None, op0=mult,
            )
            bv_sb = singles.tile([1, H], f32)
            nc.scalar.dma_start(out=bv_sb, in_=bv_ext[None, :])

            # Preload the exp ACT table while ScalarE is idle.
            tbl_dummy = singles.tile([P, 1], f32)
            nc.scalar.activation(out=tbl_dummy, in_=ident_sb[:, 0:1], func=Exp)

            # ---- per-batch prologue state ----
            xn_tiles = [[None] * 4 for _ in range(BPC)]  # [b][k] -> [P, 4, 512]
            qkTA = [None] * BPC
            qkTB = [None] * BPC
            v_sbs = {0: [None] * 4, 1: [None] * 4}  # [b][g] -> [P, 4, H] bf16

            def cast_quarters(b, ks):
                """cast HBM f32 -> SBUF xn bf16 (SWDGE queue)."""
                for k in ks:
                    xn_bk = xn_pool.tile([P, 4, 512], bf16, tag="xn",
                                         name=f"xn{b}_{k}")
                    xv = x_ext[b, 512 * k : 512 * (k + 1), :].rearrange(
                        "(t p) d -> p t d", p=P
                    )
                    nc.gpsimd.dma_start(out=xn_bk, in_=xv)
                    xn_tiles[b][k] = xn_bk

            def pe_transpose_j(b, h, j, evac):
                """xT for one (s-half, d-block): 8 PE transposes + evac."""
                tp = mm_pool.tile([P, 1024], bf16, tag="mm",
                                  name=f"tp{b}_{j}_{h}")
                for u in range(8):
                    k = 2 * h + u // 4
                    tt = u % 4
                    nc.tensor.transpose(
                        out=tp[:, u * P : (u + 1) * P],
                        in_=xn_tiles[b][k][:, tt, j * P : (j + 1) * P],
                        identity=ident_b,
                    )
                xT_t = xT_pool.tile([P, 1024], bf16, tag="xT",
                                    name=f"xT{b}_{j}_{h}")
                if evac == "scalar":
                    nc.scalar.activation(out=xT_t[:, 0:512], in_=tp[:, 0:512],
                                         func=Copy)
                    nc.scalar.activation(out=xT_t[:, 512:1024],
                                         in_=tp[:, 512:1024], func=Copy)
                else:
                    nc.vector.tensor_copy(out=xT_t, in_=tp)
                return xT_t

            def pe_transpose_half(b, h, evac):
                return {j: pe_transpose_j(b, h, j, evac) for j in range(ND)}

            def proj_chunks(b, cs, xTs, evac_engine):
                """projection chunks: all A matmuls first, then permute-B."""
                if qkTA[b] is None:
                    qkTA[b] = qkT_pool.tile([P, S], bf16, tag="qkT", name=f"qkTA{b}")
                    qkTB[b] = qkT_pool.tile([P, S], bf16, tag="qkT", name=f"qkTB{b}")
                A, Bt = qkTA[b], qkTB[b]
                for c in cs:
                    sl = slice(c * 512, (c + 1) * 512)
                    ssl = slice((c % 2) * 512, (c % 2) * 512 + 512)
                    pa = epi_pool.tile([P, 512], f32, tag="epi", name=f"pa{b}_{c}")
                    for j in range(ND):
                        nc.tensor.matmul(
                            pa, lhsT=wA[:, j, :], rhs=xTs[j][:, ssl],
                            start=(j == 0), stop=(j == ND - 1),
                        )
                    if evac_engine == "scalar":
                        nc.scalar.activation(
                            out=A[:, sl], in_=pa, func=Identity, bias=biasA[:, 0:1]
                        )
                    else:
                        nc.vector.tensor_scalar(
                            out=A[:, sl], in0=pa,
                            scalar1=biasA[:, 0:1], scalar2=None, op0=add,
                        )
                for c in cs:
                    sl = slice(c * 512, (c + 1) * 512)
                    pb = epi_pool.tile([P, 512], f32, tag="epi", name=f"pb{b}_{c}")
                    nc.tensor.matmul(pb, lhsT=perm_sb, rhs=A[:, sl],
                                     start=True, stop=True)
                    if evac_engine == "scalar":
                        nc.scalar.activation(out=Bt[:, sl], in_=pb, func=Copy)
                    else:
                        nc.vector.tensor_copy(out=Bt[:, sl], in_=pb)

            def proj_v_h(b, h, xTs):
                """v blocks [128t, 64h] for one s-half: vT = Wv^T xT then
                PE transposes to [t-part, h] groups."""
                vT_sb = vt_pool.tile([H, 1024], bf16, tag="vt",
                                     name=f"vT{b}_{h}")
                for half2 in range(2):
                    vsl = slice(half2 * 512, half2 * 512 + 512)
                    vp = epi_pool.tile([H, 512], f32, tag="epi",
                                       name=f"vp{b}_{h}_{half2}")
                    for j in range(ND):
                        nc.tensor.matmul(
                            vp, lhsT=wv_b[:, j, :], rhs=xTs[j][:, vsl],
                            start=(j == 0), stop=(j == ND - 1),
                        )
                    nc.vector.tensor_copy(out=vT_sb[:, vsl], in_=vp)
                for gg in range(2):
                    g = 2 * h + gg
                    vps = epi_pool.tile([P, 4, H], bf16, tag="epi",
                                        name=f"vps{b}_{g}")
                    for u in range(4):
                        loc = 4 * gg + u
                        nc.tensor.transpose(
                            out=vps[:, u, :],
                            in_=vT_sb[0:H, loc * P : (loc + 1) * P],
                            identity=ident_b[0:H, 0:H],
                        )
                    v_g = vsb_pool.tile([P, 4, H], bf16, tag="v",
                                        name=f"v{b}_{g}")
                    nc.vector.tensor_copy(out=v_g, in_=vps)
                    v_sbs[b][g] = v_g

            # Per-strip storage
            w_store = {0: [None] * NT, 1: [None] * NT}
            w_rhs = {0: [None] * NT, 1: [None] * NT}

            def alloc_strip(b, i):
                if (16 * b + i) in DVE_TILES:
                    sct = sc_pool.tile([P, S], i16, tag="sc", name=f"sc{b}_{i}")
                    w_store[b][i] = sct
                    w_rhs[b][i] = sct.bitcast(bf16)
                else:
                    wt = w_pool.tile([P, S], bf16, tag="w", name=f"w{b}_{i}")
                    w_store[b][i] = wt
                    w_rhs[b][i] = wt

            # ---- scores + exp for one row strip ----
            def scores_tile(b, i, z_all, halves=(0, 1)):
                A, Bt = qkTA[b], qkTB[b]
                qsl = slice(i * P, (i + 1) * P)
                on_dve = (16 * b + i) in DVE_TILES
                wt = w_store[b][i]
                for half in halves:
                    c0, c1 = 2 * half, 2 * half + 1
                    ps = mm_pool.tile([P, 1024], f32, tag="mm",
                                      name=f"ps{b}_{i}_{half}")
                    nc.tensor.matmul(
                        ps[:, 0:512], lhsT=A[0:H, qsl],
                        rhs=Bt[0:H, c0 * 512 : (c0 + 1) * 512],
                        start=True, stop=True, tile_position=(0, 0),
                    )
                    nc.tensor.matmul(
                        ps[:, 512:1024], lhsT=Bt[H:P, qsl],
                        rhs=A[H:P, c1 * 512 : (c1 + 1) * 512],
                        start=True, stop=True, tile_position=(H, 0),
                    )
                    hs = slice(half * 1024, (half + 1) * 1024)
                    if on_dve:
                        nc.vector.tensor_scalar(
                            out=wt[:, hs], in0=ps,
                            scalar1=A16, scalar2=B16, op0=mult, op1=add,
                        )
                    else:
                        nc.scalar.activation(
                            out=wt[:, hs], in_=ps, func=Exp,
                            accum_out=z_all[:, i, half : half + 1],
                        )
                if on_dve and 1 in halves:
                    wbt = wb_pool.tile([P, S], bf16, tag="wb", name=f"wb{b}_{i}")
                    nc.sync.dma_start(out=wbt, in_=wt.bitcast(bf16))
                    nc.vector.tensor_add(
                        out=wbt[:, 0:1024], in0=wbt[:, 0:1024],
                        in1=wbt[:, 1024:2048],
                    )
                    nc.vector.reduce_sum(
                        out=z_all[:, i, 0:1], in_=wbt[:, 0:1024], axis=X
                    )

            def colsum_group(b, i0, z_all, rz_all, rzb_all, colbank, n=4):
                nc.vector.reduce_sum(
                    out=rz_all[:, i0 : i0 + n], in_=z_all[:, i0 : i0 + n, :], axis=X
                )
                nc.vector.reciprocal(
                    out=rz_all[:, i0 : i0 + n], in_=rz_all[:, i0 : i0 + n]
                )
                nc.vector.tensor_copy(
                    out=rzb_all[:, i0 : i0 + n], in_=rz_all[:, i0 : i0 + n]
                )
                for i in range(i0, i0 + n):
                    for c in range(NC4):
                        nc.tensor.matmul(
                            colbank[32 * c : 32 * c + 1, :],
                            lhsT=rzb_all[:, i : i + 1],
                            rhs=w_rhs[b][i][:, c * 512 : (c + 1) * 512],
                            start=(i == 0), stop=(i == NT - 1),
                            tile_position=(0, 32 * c),
                        )

            def epilogue_a(b, colbank):
                """cbar evac + transpose + extract (DVE/PE round trip)."""
                cbar_sb = misc_pool.tile([P, 512], f32, tag="cbar", name=f"cbar{b}")
                nc.vector.tensor_copy(out=cbar_sb, in_=colbank)
                cbT_ps = epi_pool.tile([P, 512], f32, tag="epi", name=f"cbT{b}")
                for f in range(4):
                    nc.tensor.transpose(
                        out=cbT_ps[:, f * P : (f + 1) * P],
                        in_=cbar_sb[:, f * P : (f + 1) * P],
                        identity=ident_sb,
                    )
                cbT_sb = misc_pool.tile([P, 4, 4], bf16, tag="cbT", name=f"cbT{b}")
                src = cbT_ps[:, :].rearrange("p (f c r) -> p c f r", f=4, c=4, r=32)
                nc.vector.tensor_copy(out=cbT_sb, in_=src[:, :, :, 0])
                return cbT_sb

            def epilogue_b(b, cbT_sb):
                fp = epi_pool.tile([1, H], f32, tag="epi", name=f"fp{b}")
                for t in range(NT):
                    nc.tensor.matmul(
                        fp, lhsT=cbT_sb[:, t // 4, t % 4 : t % 4 + 1],
                        rhs=v_sbs[b][t // 4][:, t % 4, :],
                        start=(t == 0), stop=(t == NT - 1),
                    )
                o_sb = misc_pool.tile([1, H], f32, tag="o", name=f"o{b}")
                nc.vector.scalar_tensor_tensor(
                    out=o_sb, in0=fp, scalar=1.0 / float(S), in1=bv_sb,
                    op0=mult, op1=add,
                )
                nc.sync.dma_start(out=out_ext[b : b + 1, :], in_=o_sb)

            # ================= schedule =================
            z_alls = {}
            rz_alls = {}
            rzb_alls = {}
            colbanks = {}
            for b in range(BPC):
                z_alls[b] = zr_pool.tile([P, NT, 2], f32, tag="z", name=f"z{b}")
                nc.vector.memset(z_alls[b], 0.0)
                rz_alls[b] = zr_pool.tile([P, NT], f32, tag="rz", name=f"rz{b}")
                rzb_alls[b] = zr_pool.tile([P, NT], bf16, tag="rzb", name=f"rzb{b}")

            def get_colbank(b):
                cb = col_pool.tile([P, 512], f32, tag="col", name=f"cb{b}")
                nc.vector.memset(cb, 0.0)
                colbanks[b] = cb

            # batch 0 prologue: casts up-front, PE transposes, scalar evacs
            cast_quarters(0, (0, 1))
            cast_quarters(0, (2, 3))
            xT00 = pe_transpose_half(0, 0, "scalar")
            proj_chunks(0, (0, 1), xT00, "vector")
            xT01 = pe_transpose_half(0, 1, "vector")

            get_colbank(0)
            za0 = z_alls[0]
            # only 3 strips need the h0/h1 split -- just enough to hide the
            # proj(2,3) latency; the rest run as full strips
            for i in range(3):
                alloc_strip(0, i)
                scores_tile(0, i, za0, halves=(0,))
            proj_chunks(0, (2, 3), xT01, "vector")
            for i in range(3):
                scores_tile(0, i, za0, halves=(1,))
            for i in range(3, 6):
                alloc_strip(0, i)
                scores_tile(0, i, za0)
                if i == 5:
                    colsum_group(0, 0, za0, rz_alls[0], rzb_alls[0], colbanks[0])

            cast_quarters(1, (0, 1))
            cast_quarters(1, (2, 3))

            for i in range(6, 8):
                alloc_strip(0, i)
                scores_tile(0, i, za0)
            # interleave batch-1 PE transposes (per d-block) between strips
            # so they never block strip pair matmuls in the PE queue
            xT10 = {}
            xT11 = {}
            for i in range(8, 12):
                alloc_strip(0, i)
                scores_tile(0, i, za0)
                xT10[i - 8] = pe_transpose_j(1, 0, i - 8, "vector")
                if i == 9:
                    colsum_group(0, 4, za0, rz_alls[0], rzb_alls[0], colbanks[0])

            for i in range(12, NT):
                alloc_strip(0, i)
                scores_tile(0, i, za0)
                xT11[i - 12] = pe_transpose_j(1, 1, i - 12, "vector")
                if i == 13:
                    colsum_group(0, 8, za0, rz_alls[0], rzb_alls[0], colbanks[0])
                    proj_chunks(1, (0, 1), xT10, "vector")

            proj_chunks(1, (2, 3), xT11, "vector")

            colsum_group(0, 12, z_alls[0], rz_alls[0], rzb_alls[0],
                         colbanks[0], n=2)
            colsum_group(0, 14, z_alls[0], rz_alls[0], rzb_alls[0],
                         colbanks[0], n=2)
            cbT0 = epilogue_a(0, colbanks[0])

            # batch-1: all proj chunks are ready before its first strip, so
            # run FULL strips (the b0-style halves-split only exists to hide
            # proj latency) with v/epilogue pieces spread thinly between them
            get_colbank(1)
            za1 = z_alls[1]
            for i in range(NT):
                alloc_strip(1, i)
                scores_tile(1, i, za1)
                if i == 1:
                    proj_v_h(1, 0, xT10)
                if i == 3:
                    proj_v_h(0, 0, xT00)
                if i == 5:
                    proj_v_h(0, 1, xT01)
                    colsum_group(1, 0, za1, rz_alls[1], rzb_alls[1], colbanks[1])
                if i == 7:
                    epilogue_b(0, cbT0)
                if i == 9:
                    colsum_group(1, 4, za1, rz_alls[1], rzb_alls[1], colbanks[1])
                if i == 10:
                    proj_v_h(1, 1, xT11)
                if i == 13:
                    colsum_group(1, 8, za1, rz_alls[1], rzb_alls[1], colbanks[1])
                if i == 14:
                    colsum_group(1, 12, za1, rz_alls[1], rzb_alls[1],
                                 colbanks[1], n=2)
            colsum_group(1, 14, z_alls[1], rz_alls[1], rzb_alls[1],
                         colbanks[1], n=2)
            cbT1 = epilogue_a(1, colbanks[1])
            epilogue_b(1, cbT1)

    nc.finalize()
    return nc


_NC_CACHE = None


def _get_nc():
    global _NC_CACHE
    if _NC_CACHE is None:
        _NC_CACHE = build_nc()
    return _NC_CACHE


def run(inputs_map, trace=False, **spmd_kwargs):
    from concourse.bass_utils import run_bass_kernel_spmd

    x = np.ascontiguousarray(np.asarray(inputs_map["inputs"], dtype=np.float32))
    assert x.shape == (B, S, D), x.shape
    full = {
        "Wq": np.ascontiguousarray(np.asarray(inputs_map["Wq"], np.float32)),
        "bq": np.ascontiguousarray(np.asarray(inputs_map["bq"], np.float32)),
        "Wk": np.ascontiguousarray(np.asarray(inputs_map["Wk"], np.float32)),
        "bk": np.ascontiguousarray(np.asarray(inputs_map["bk"], np.float32)),
        "Wv": np.ascontiguousarray(np.asarray(inputs_map["Wv"], np.float32)),
        "bv": np.ascontiguousarray(np.asarray(inputs_map["bv"], np.float32)),
        "ident128": np.eye(P, dtype=np.float32),
        "perm64": np.roll(np.eye(P, dtype=np.float32), 64, axis=0),
    }
    in_maps = []
    for i in range(NCORES):
        m = {"inputs": np.ascontiguousarray(x[i * BPC : (i + 1) * BPC])}
        m.update(full)
        in_maps.append(m)
    nc = _get_nc()
    res = run_bass_kernel_spmd(
        nc, in_maps, core_ids=list(range(NCORES)), trace=trace, **spmd_kwargs
    )
    out = np.concatenate([np.asarray(res.results[i]["out"]) for i in range(NCORES)], 0)
    return out.astype(np.float32), res


def kernel(**inputs):
    out, _ = run(inputs, trace=False)
    return out


if __name__ == "__main__":
    rng = np.random.default_rng(0)
    ins = {
        "inputs": rng.standard_normal((B, S, D), dtype=np.float32),
        "Wq": rng.standard_normal((D, H), dtype=np.float32) / np.sqrt(D),
        "bq": np.zeros(H, np.float32),
        "Wk": rng.standard_normal((D, H), dtype=np.float32) / np.sqrt(D),
        "bk": np.zeros(H, np.float32),
        "Wv": rng.standard_normal((D, H), dtype=np.float32) / np.sqrt(D),
        "bv": np.zeros(H, np.float32),
    }
    out = kernel(**ins)
    print("out", out.shape, out[0, :4])


# revision 25
# speedup vs baseline: 1.0306x; 1.0190x over previous
"""Trainium2 Bass kernel for nn_AttentionLayer (B=16, S=2048, D=512, H=64).

Data-parallel over batch: 8 NeuronCores x 2 batch items each; no collectives.

Math (per batch item b):
  qT = (Wq^T x^T + bq)/sqrt(H);  kT = Wk^T x^T + bk      [64, S] each
  w = exp(qT^T kT)  (no rowmax pass: |scores| <= ~9)
  out[h] = sum_t cbar[t] * v[t, :] / S + bv, cbar[t] = sum_s w[s,t]/Z[s]
  (uses sum_t cbar[t] == S so bv needs no extra scaling)

Structure:
  - ALL x transposes on the PE (no DRAM bounce, no DMA transposes):
    SWDGE casts HBM f32 -> SBUF xn bf16, then per-half groups of 32 PE
    transposes (bf16 identity, bf16 PSUM) evacuated to xT bf16 tiles by
    ScalarE (batch 0, idle prologue) / VectorE (batch 1, mid-strips).
    PE transposes double as HAM warmup.
  - fused projection stationary [Wq/sqrt(H) | Wk] -> A = [qT;kT];
    B = [kT;qT] via one PE permutation matmul per chunk.
  - scores row-strips [128,2048] via row-packed matmul pairs
    (tile_position (0,0)/(64,0)) streaming two column chunks at once.
  - exp SPLIT between ScalarE and VectorE per row-strip:
      * ScalarE strips: activation Exp (f32 PSUM -> bf16 SBUF) with
        accum_out rowsum.
      * DVE strips: Schraudolph bit-trick (i16 = s*log2e*128 + 16250.9 ==
        bf16 bits of exp(s)) via tensor_scalar f32 PSUM -> i16 SBUF.
        The colsum matmul reads the i16 tile DIRECTLY via .bitcast(bf16)
        (PE rhs reads of bitcast APs are safe on HW - measured; DVE and
        ScalarE compute ops on bitcast APs crash).  Rowsum: one byte-copy
        (sync HWDGE queue, transpose-free so no mode-switch hazards) to a
        real bf16 tile + tensor_add fold 2048->1024 + one 1x reduce_sum.
        accum_out on DVE is avoided (its CACHE_REDUCE costs a full 1x
        pass, ~2.3us).  DVE strips sit in colsum groups 0-2 only so the
        last group never waits on the byte-copy chain.
      Per-row softmax normalization cancels the Schraudolph per-element
      bias exactly (each row lives entirely on one engine).
  - colsum in groups of 4 strips (batched reciprocal), col-packed
    (0,32c) accumulation into one psum bank.
  - epilogue via V tiles: vT = Wv^T xT (PE), PE-transposed to v blocks
    [128t, 64h]; out = sum_t cbT[t] v[t]  -- 64-col streams instead of
    the old 512-col x-based streams, and no g/gT round-trips.
  - epilogue split in two parts with batch-1 strips issued in between so
    PE/ScalarE stay fed across the batch boundary.

Known landmines (measured on HW):
  - bitcast APs on DVE/ScalarE tensor ops -> NRT_EXEC_UNIT_UNRECOVERABLE.
    PE matmul rhs and DMA APs are safe.
  - matmul output must be fp32 (bass asserts); transpose output must
    match the input dtype (bf16 transposes write bf16 PSUM).
  - SWDGE DRAM->DRAM cast DMA -> INTERNAL crash.
  - gpsimd tensor_scalar with accum_out fails to compile; accum_out
    needs op1 set; gpsimd cannot read PSUM or reduce along free axis.
  - interleaving SBUF->SBUF copies with DMA transposes on the Sync queue
    causes multi-us mode-switch drains (moot now: no DMA transposes).
"""

import os as _os

import numpy as np

B, S, D, H = 16, 2048, 512, 64
NCORES = 8
BPC = B // NCORES  # batches per core
P = 128
NT = S // P  # 16 row strips
ND = D // P  # 4 d tiles
NC4 = S // 512  # 4 column chunks of 512

# Row strips whose exp runs on VectorE (Schraudolph); rest on ScalarE.
if _os.environ.get("K_DVE_EXP"):
    DVE_TILES = tuple(int(t) for t in _os.environ["K_DVE_EXP"].split(","))
else:
    _per = tuple(
        int(t) for t in _os.environ.get("K_DVE_PER", "1,3,6,9,11").split(",")
    )
    DVE_TILES = tuple(b * 16 + i for b in range(BPC) for i in _per)

# Schraudolph constants for bf16-bits exp: i16 = s * A16 + B16
A16 = float(np.log2(np.e) * 128.0)
B16 = 16250.91

N_WARMUP_MM = int(_os.environ.get("K_WARM", "48"))


def build_nc():
    import concourse.bacc as bacc
    import concourse.mybir as mybir
    import concourse.tile as tile

    f32 = mybir.dt.float32
    bf16 = mybir.dt.bfloat16
    i16 = mybir.dt.int16
    Exp = mybir.ActivationFunctionType.Exp
    Identity = mybir.ActivationFunctionType.Identity
    Copy = mybir.ActivationFunctionType.Copy
    X = mybir.AxisListType.X
    add = mybir.AluOpType.add
    mult = mybir.AluOpType.mult

    nc = bacc.Bacc("TRN2", target_bir_lowering=False)

    x_ext = nc.declare_dram_parameter("inputs", [BPC, S, D], f32, isOutput=False)
    wq_ext = nc.declare_dram_parameter("Wq", [D, H], f32, isOutput=False)
    bq_ext = nc.declare_dram_parameter("bq", [H], f32, isOutput=False)
    wk_ext = nc.declare_dram_parameter("Wk", [D, H], f32, isOutput=False)
    bk_ext = nc.declare_dram_parameter("bk", [H], f32, isOutput=False)
    wv_ext = nc.declare_dram_parameter("Wv", [D, H], f32, isOutput=False)
    bv_ext = nc.declare_dram_parameter("bv", [H], f32, isOutput=False)
    ident_ext = nc.declare_dram_parameter("ident128", [P, P], f32, isOutput=False)
    perm_ext = nc.declare_dram_parameter("perm64", [P, P], f32, isOutput=False)
    out_ext = nc.declare_dram_parameter("out", [BPC, H], f32, isOutput=True)

    inv_sqrt_h = 1.0 / float(np.sqrt(H))

    with tile.TileContext(nc) as tc:
        with (
            tc.tile_pool(name="singles", bufs=1) as singles,
            tc.tile_pool(name="xn", bufs=8) as xn_pool,
            tc.tile_pool(name="xT", bufs=16) as xT_pool,
            tc.tile_pool(name="qkT", bufs=4) as qkT_pool,
            tc.tile_pool(name="w", bufs=9) as w_pool,
            tc.tile_pool(name="sc16", bufs=4) as sc_pool,
            tc.tile_pool(name="wb", bufs=3) as wb_pool,
            tc.tile_pool(name="vt", bufs=2) as vt_pool,
            tc.tile_pool(name="vsb", bufs=8) as vsb_pool,
            tc.tile_pool(name="zr", bufs=6) as zr_pool,
            tc.tile_pool(name="misc", bufs=4) as misc_pool,
            tc.tile_pool(name="mm", bufs=3, space="PSUM") as mm_pool,
            tc.tile_pool(name="col", bufs=1, space="PSUM") as col_pool,
            tc.tile_pool(name="epi", bufs=1, space="PSUM") as epi_pool,
        ):
            # ---- PE warmup first: a memset tile needs no DMA, so the
            # HAM clock-gate release clock starts at ~0.3us
            warm_sb = singles.tile([P, P], bf16)
            nc.vector.memset(warm_sb, 0.001)
            warm_ps = epi_pool.tile([P, 512], f32, tag="epi", name="warm")
            for wi in range(N_WARMUP_MM):
                nc.tensor.matmul(
                    warm_ps[:, 0:P], lhsT=warm_sb, rhs=warm_sb,
                    start=True, stop=True,
                )

            # ---- constants / weights prep (once); scalar HWDGE queue ----
            ident_sb = singles.tile([P, P], f32)
            nc.scalar.dma_start(out=ident_sb, in_=ident_ext[:, :])
            perm_f = singles.tile([P, P], f32)
            nc.scalar.dma_start(out=perm_f, in_=perm_ext[:, :])
            perm_sb = singles.tile([P, P], bf16)
            nc.vector.tensor_copy(out=perm_sb, in_=perm_f)
            ident_b = singles.tile([P, P], bf16)
            nc.vector.tensor_copy(out=ident_b, in_=ident_sb)

            wq_f = singles.tile([P, ND, H], f32)
            nc.scalar.dma_start(out=wq_f, in_=wq_ext.rearrange("(j p) h -> p j h", p=P))
            wk_f = singles.tile([P, ND, H], f32)
            nc.scalar.dma_start(out=wk_f, in_=wk_ext.rearrange("(j p) h -> p j h", p=P))
            wv_f = singles.tile([P, ND, H], f32)
            nc.scalar.dma_start(out=wv_f, in_=wv_ext.rearrange("(j p) h -> p j h", p=P))

            # Fused projection stationary: wA = [Wq/sqrt(H) | Wk] per d-chunk
            wA = singles.tile([P, ND, P], bf16)
            for j in range(ND):
                nc.vector.tensor_scalar(
                    out=wA[:, j, 0:H], in0=wq_f[:, j, :],
                    scalar1=inv_sqrt_h, scalar2=None, op0=mult,
                )
                nc.vector.tensor_copy(out=wA[:, j, H:P], in_=wk_f[:, j, :])
            wv_b = singles.tile([P, ND, H], bf16)
            for j in range(ND):
                nc.vector.tensor_copy(out=wv_b[:, j, :], in_=wv_f[:, j, :])

            # biasA: rows 0:64 = bq/sqrt(H), rows 64:128 = bk
            biasA = singles.tile([P, 1], f32)
            nc.scalar.dma_start(out=biasA[0:H, 0:1], in_=bq_ext[:, None])
            nc.scalar.dma_start(out=biasA[H:P, 0:1], in_=bk_ext[:, None])
            nc.vector.tensor_scalar(
                out=biasA[0:H, 0:1], in0=biasA[0:H, 0:1],
                scalar1=inv_sqrt_h, scalar2=None, op0=mult,
            )
            bv_sb = singles.tile([1, H], f32)
            nc.scalar.dma_start(out=bv_sb, in_=bv_ext[None, :])

            # Preload the exp ACT table while ScalarE is idle.
            tbl_dummy = singles.tile([P, 1], f32)
            nc.scalar.activation(out=tbl_dummy, in_=ident_sb[:, 0:1], func=Exp)

            # ---- per-batch prologue state ----
            xn_tiles = [[None] * 4 for _ in range(BPC)]  # [b][k] -> [P, 4, 512]
            qkTA = [None] * BPC
            qkTB = [None] * BPC
            v_sbs = {0: [None] * 4, 1: [None] * 4}  # [b][g] -> [P, 4, H] bf16

            def cast_quarters(b, ks):
                """cast HBM f32 -> SBUF xn bf16 (SWDGE queue)."""
                for k in ks:
                    xn_bk = xn_pool.tile([P, 4, 512], bf16, tag="xn",
                                         name=f"xn{b}_{k}")
                    xv = x_ext[b, 512 * k : 512 * (k + 1), :].rearrange(
                        "(t p) d -> p t d", p=P
                    )
                    nc.gpsimd.dma_start(out=xn_bk, in_=xv)
                    xn_tiles[b][k] = xn_bk

            def pe_transpose_j(b, h, j, evac):
                """xT for one (s-half, d-block): 8 PE transposes + evac."""
                tp = mm_pool.tile([P, 1024], bf16, tag="mm",
                                  name=f"tp{b}_{j}_{h}")
                for u in range(8):
                    k = 2 * h + u // 4
                    tt = u % 4
                    nc.tensor.transpose(
                        out=tp[:, u * P : (u + 1) * P],
                        in_=xn_tiles[b][k][:, tt, j * P : (j + 1) * P],
                        identity=ident_b,
                    )
                xT_t = xT_pool.tile([P, 1024], bf16, tag="xT",
                                    name=f"xT{b}_{j}_{h}")
                if evac == "scalar":
                    nc.scalar.activation(out=xT_t[:, 0:512], in_=tp[:, 0:512],
                                         func=Copy)
                    nc.scalar.activation(out=xT_t[:, 512:1024],
                                         in_=tp[:, 512:1024], func=Copy)
                else:
                    nc.vector.tensor_copy(out=xT_t, in_=tp)
                return xT_t

            def pe_transpose_half(b, h, evac):
                return {j: pe_transpose_j(b, h, j, evac) for j in range(ND)}

            def proj_chunks(b, cs, xTs, evac_engine):
                """projection chunks: all A matmuls first, then permute-B."""
                if qkTA[b] is None:
                    qkTA[b] = qkT_pool.tile([P, S], bf16, tag="qkT", name=f"qkTA{b}")
                    qkTB[b] = qkT_pool.tile([P, S], bf16, tag="qkT", name=f"qkTB{b}")
                A, Bt = qkTA[b], qkTB[b]
                for c in cs:
                    sl = slice(c * 512, (c + 1) * 512)
                    ssl = slice((c % 2) * 512, (c % 2) * 512 + 512)
                    pa = epi_pool.tile([P, 512], f32, tag="epi", name=f"pa{b}_{c}")
                    for j in range(ND):
                        nc.tensor.matmul(
                            pa, lhsT=wA[:, j, :], rhs=xTs[j][:, ssl],
                            start=(j == 0), stop=(j == ND - 1),
                        )
                    if evac_engine == "scalar":
                        nc.scalar.activation(
                            out=A[:, sl], in_=pa, func=Identity, bias=biasA[:, 0:1]
                        )
                    else:
                        nc.vector.tensor_scalar(
                            out=A[:, sl], in0=pa,
                            scalar1=biasA[:, 0:1], scalar2=None, op0=add,
                        )
                for c in cs:
                    sl = slice(c * 512, (c + 1) * 512)
                    pb = epi_pool.tile([P, 512], f32, tag="epi", name=f"pb{b}_{c}")
                    nc.tensor.matmul(pb, lhsT=perm_sb, rhs=A[:, sl],
                                     start=True, stop=True)
                    if evac_engine == "scalar":
                        nc.scalar.activation(out=Bt[:, sl], in_=pb, func=Copy)
                    else:
                        nc.vector.tensor_copy(out=Bt[:, sl], in_=pb)

            def proj_v_h(b, h, xTs):
                """v blocks [128t, 64h] for one s-half: vT = Wv^T xT then
                PE transposes to [t-part, h] groups."""
                vT_sb = vt_pool.tile([H, 1024], bf16, tag="vt",
                                     name=f"vT{b}_{h}")
                for half2 in range(2):
                    vsl = slice(half2 * 512, half2 * 512 + 512)
                    vp = epi_pool.tile([H, 512], f32, tag="epi",
                                       name=f"vp{b}_{h}_{half2}")
                    for j in range(ND):
                        nc.tensor.matmul(
                            vp, lhsT=wv_b[:, j, :], rhs=xTs[j][:, vsl],
                            start=(j == 0), stop=(j == ND - 1),
                        )
                    nc.vector.tensor_copy(out=vT_sb[:, vsl], in_=vp)
                for gg in range(2):
                    g = 2 * h + gg
                    vps = epi_pool.tile([P, 4, H], bf16, tag="epi",
                                        name=f"vps{b}_{g}")
                    for u in range(4):
                        loc = 4 * gg + u
                        nc.tensor.transpose(
                            out=vps[:, u, :],
                            in_=vT_sb[0:H, loc * P : (loc + 1) * P],
                            identity=ident_b[0:H, 0:H],
                        )
                    v_g = vsb_pool.tile([P, 4, H], bf16, tag="v",
                                        name=f"v{b}_{g}")
                    nc.vector.tensor_copy(out=v_g, in_=vps)
                    v_sbs[b][g] = v_g

            # Per-strip storage
            w_store = {0: [None] * NT, 1: [None] * NT}
            w_rhs = {0: [None] * NT, 1: [None] * NT}

            def alloc_strip(b, i):
                if (16 * b + i) in DVE_TILES:
                    sct = sc_pool.tile([P, S], i16, tag="sc", name=f"sc{b}_{i}")
                    w_store[b][i] = sct
                    w_rhs[b][i] = sct.bitcast(bf16)
                else:
                    wt = w_pool.tile([P, S], bf16, tag="w", name=f"w{b}_{i}")
                    w_store[b][i] = wt
                    w_rhs[b][i] = wt

            # ---- scores + exp for one row strip ----
            def scores_tile(b, i, z_all, halves=(0, 1)):
                A, Bt = qkTA[b], qkTB[b]
                qsl = slice(i * P, (i + 1) * P)
                on_dve = (16 * b + i) in DVE_TILES
                wt = w_store[b][i]
                for half in halves:
                    c0, c1 = 2 * half, 2 * half + 1
                    ps = mm_pool.tile([P, 1024], f32, tag="mm",
                                      name=f"ps{b}_{i}_{half}")
                    nc.tensor.matmul(
                        ps[:, 0:512], lhsT=A[0:H, qsl],
                        rhs=Bt[0:H, c0 * 512 : (c0 + 1) * 512],
                        start=True, stop=True, tile_position=(0, 0),
                    )
                    nc.tensor.matmul(
                        ps[:, 512:1024], lhsT=Bt[H:P, qsl],
                        rhs=A[H:P, c1 * 512 : (c1 + 1) * 512],
                        start=True, stop=True, tile_position=(H, 0),
                    )
                    hs = slice(half * 1024, (half + 1) * 1024)
                    if on_dve:
                        nc.vector.tensor_scalar(
                            out=wt[:, hs], in0=ps,
                            scalar1=A16, scalar2=B16, op0=mult, op1=add,
                        )
                    else:
                        nc.scalar.activation(
                            out=wt[:, hs], in_=ps, func=Exp,
                            accum_out=z_all[:, i, half : half + 1],
                        )
                if on_dve and 1 in halves:
                    wbt = wb_pool.tile([P, S], bf16, tag="wb", name=f"wb{b}_{i}")
                    nc.sync.dma_start(out=wbt, in_=wt.bitcast(bf16))
                    nc.vector.tensor_add(
                        out=wbt[:, 0:1024], in0=wbt[:, 0:1024],
                        in1=wbt[:, 1024:2048],
                    )
                    nc.vector.reduce_sum(
                        out=z_all[:, i, 0:1], in_=wbt[:, 0:1024], axis=X
                    )

            def colsum_group(b, i0, z_all, rz_all, rzb_all, colbank, n=4):
                nc.vector.reduce_sum(
                    out=rz_all[:, i0 : i0 + n], in_=z_all[:, i0 : i0 + n, :], axis=X
                )
                nc.vector.reciprocal(
                    out=rz_all[:, i0 : i0 + n], in_=rz_all[:, i0 : i0 + n]
                )
                nc.vector.tensor_copy(
                    out=rzb_all[:, i0 : i0 + n], in_=rz_all[:, i0 : i0 + n]
                )
                for i in range(i0, i0 + n):
                    for c in range(NC4):
                        nc.tensor.matmul(
                            colbank[32 * c : 32 * c + 1, :],
                            lhsT=rzb_all[:, i : i + 1],
                            rhs=w_rhs[b][i][:, c * 512 : (c + 1) * 512],
                            start=(i == 0), stop=(i == NT - 1),
                            tile_position=(0, 32 * c),
                        )

            def epilogue_a(b, colbank):
                """cbar evac + transpose + extract (DVE/PE round trip)."""
                cbar_sb = misc_pool.tile([P, 512], f32, tag="cbar", name=f"cbar{b}")
                nc.vector.tensor_copy(out=cbar_sb, in_=colbank)
                cbT_ps = epi_pool.tile([P, 512], f32, tag="epi", name=f"cbT{b}")
                for f in range(4):
                    nc.tensor.transpose(
                        out=cbT_ps[:, f * P : (f + 1) * P],
                        in_=cbar_sb[:, f * P : (f + 1) * P],
                        identity=ident_sb,
                    )
                cbT_sb = misc_pool.tile([P, 4, 4], bf16, tag="cbT", name=f"cbT{b}")
                src = cbT_ps[:, :].rearrange("p (f c r) -> p c f r", f=4, c=4, r=32)
                nc.vector.tensor_copy(out=cbT_sb, in_=src[:, :, :, 0])
                return cbT_sb

            def epilogue_b(b, cbT_sb):
                fp = epi_pool.tile([1, H], f32, tag="epi", name=f"fp{b}")
                for t in range(NT):
                    nc.tensor.matmul(
                        fp, lhsT=cbT_sb[:, t // 4, t % 4 : t % 4 + 1],
                        rhs=v_sbs[b][t // 4][:, t % 4, :],
                        start=(t == 0), stop=(t == NT - 1),
                    )
                o_sb = misc_pool.tile([1, H], f32, tag="o", name=f"o{b}")
                nc.vector.scalar_tensor_tensor(
                    out=o_sb, in0=fp, scalar=1.0 / float(S), in1=bv_sb,
                    op0=mult, op1=add,
                )
                nc.sync.dma_start(out=out_ext[b : b + 1, :], in_=o_sb)

            # ================= schedule =================
            z_alls = {}
            rz_alls = {}
            rzb_alls = {}
            colbanks = {}
            for b in range(BPC):
                z_alls[b] = zr_pool.tile([P, NT, 2], f32, tag="z", name=f"z{b}")
                nc.vector.memset(z_alls[b], 0.0)
                rz_alls[b] = zr_pool.tile([P, NT], f32, tag="rz", name=f"rz{b}")
                rzb_alls[b] = zr_pool.tile([P, NT], bf16, tag="rzb", name=f"rzb{b}")

            def get_colbank(b):
                cb = col_pool.tile([P, 512], f32, tag="col", name=f"cb{b}")
                nc.vector.memset(cb, 0.0)
                colbanks[b] = cb

            # batch 0 prologue: casts up-front, PE transposes, scalar evacs
            cast_quarters(0, (0, 1))
            cast_quarters(0, (2, 3))
            xT00 = pe_transpose_half(0, 0, "scalar")
            proj_chunks(0, (0, 1), xT00, "vector")
            xT01 = pe_transpose_half(0, 1, "vector")

            get_colbank(0)
            za0 = z_alls[0]
            for i in range(8):
                alloc_strip(0, i)
                scores_tile(0, i, za0, halves=(0,))
            proj_chunks(0, (2, 3), xT01, "vector")
            for i in range(6):
                scores_tile(0, i, za0, halves=(1,))
                if i == 5:
                    colsum_group(0, 0, za0, rz_alls[0], rzb_alls[0], colbanks[0])

            cast_quarters(1, (0, 1))
            cast_quarters(1, (2, 3))

            for i in range(6, 8):
                scores_tile(0, i, za0, halves=(1,))
            # interleave batch-1 PE transposes (per d-block) between strips
            # so they never block strip pair matmuls in the PE queue
            xT10 = {}
            xT11 = {}
            for i in range(8, 12):
                alloc_strip(0, i)
                scores_tile(0, i, za0)
                xT10[i - 8] = pe_transpose_j(1, 0, i - 8, "vector")
                if i == 9:
                    colsum_group(0, 4, za0, rz_alls[0], rzb_alls[0], colbanks[0])

            for i in range(12, NT):
                alloc_strip(0, i)
                scores_tile(0, i, za0)
                xT11[i - 12] = pe_transpose_j(1, 1, i - 12, "vector")
                if i == 13:
                    colsum_group(0, 8, za0, rz_alls[0], rzb_alls[0], colbanks[0])
                    proj_chunks(1, (0, 1), xT10, "vector")

            proj_chunks(1, (2, 3), xT11, "vector")

            colsum_group(0, 12, z_alls[0], rz_alls[0], rzb_alls[0],
                         colbanks[0], n=2)
            colsum_group(0, 14, z_alls[0], rz_alls[0], rzb_alls[0],
                         colbanks[0], n=2)
            cbT0 = epilogue_a(0, colbanks[0])

            # batch-1: all proj chunks are ready before its first strip, so
            # run FULL strips (the b0-style halves-split only exists to hide
            # proj latency) with v/epilogue pieces spread thinly between them
            get_colbank(1)
            za1 = z_alls[1]
            for i in range(NT):
                alloc_strip(1, i)
                scores_tile(1, i, za1)
                if i == 1:
                    proj_v_h(1, 0, xT10)
                if i == 3:
                    proj_v_h(0, 0, xT00)
                if i == 5:
                    proj_v_h(0, 1, xT01)
                    colsum_group(1, 0, za1, rz_alls[1], rzb_alls[1], colbanks[1])
                if i == 7:
                    epilogue_b(0, cbT0)
                if i == 9:
                    colsum_group(1, 4, za1, rz_alls[1], rzb_alls[1], colbanks[1])
                if i == 10:
                    proj_v_h(1, 1, xT11)
                if i == 13:
                    colsum_group(1, 8, za1, rz_alls[1], rzb_alls[1], colbanks[1])
                if i == 14:
                    colsum_group(1, 12, za1, rz_alls[1], rzb_alls[1],
                                 colbanks[1], n=2)
            colsum_group(1, 14, z_alls[1], rz_alls[1], rzb_alls[1],
                         colbanks[1], n=2)
            cbT1 = epilogue_a(1, colbanks[1])
            epilogue_b(1, cbT1)

    nc.finalize()
    return nc


_NC_CACHE = None


def _get_nc():
    global _NC_CACHE
    if _NC_CACHE is None:
        _NC_CACHE = build_nc()
    return _NC_CACHE


def run(inputs_map, trace=False, **spmd_kwargs):
    from concourse.bass_utils import run_bass_kernel_spmd

    x = np.ascontiguousarray(np.asarray(inputs_map["inputs"], dtype=np.float32))
    assert x.shape == (B, S, D), x.shape
    full = {
        "Wq": np.ascontiguousarray(np.asarray(inputs_map["Wq"], np.float32)),
        "bq": np.ascontiguousarray(np.asarray(inputs_map["bq"], np.float32)),
        "Wk": np.ascontiguousarray(np.asarray(inputs_map["Wk"], np.float32)),
        "bk": np.ascontiguousarray(np.asarray(inputs_map["bk"], np.float32)),
        "Wv": np.ascontiguousarray(np.asarray(inputs_map["Wv"], np.float32)),
        "bv": np.ascontiguousarray(np.asarray(inputs_map["bv"], np.float32)),
        "ident128": np.eye(P, dtype=np.float32),
        "perm64": np.roll(np.eye(P, dtype=np.float32), 64, axis=0),
    }
    in_maps = []
    for i in range(NCORES):
        m = {"inputs": np.ascontiguousarray(x[i * BPC : (i + 1) * BPC])}
        m.update(full)
        in_maps.append(m)
    nc = _get_nc()
    res = run_bass_kernel_spmd(
        nc, in_maps, core_ids=list(range(NCORES)), trace=trace, **spmd_kwargs
    )
    out = np.concatenate([np.asarray(res.results[i]["out"]) for i in range(NCORES)], 0)
    return out.astype(np.float32), res


def kernel(**inputs):
    out, _ = run(inputs, trace=False)
    return out


if __name__ == "__main__":
    rng = np.random.default_rng(0)
    ins = {
        "inputs": rng.standard_normal((B, S, D), dtype=np.float32),
        "Wq": rng.standard_normal((D, H), dtype=np.float32) / np.sqrt(D),
        "bq": np.zeros(H, np.float32),
        "Wk": rng.standard_normal((D, H), dtype=np.float32) / np.sqrt(D),
        "bk": np.zeros(H, np.float32),
        "Wv": rng.standard_normal((D, H), dtype=np.float32) / np.sqrt(D),
        "bv": np.zeros(H, np.float32),
    }
    out = kernel(**ins)
    print("out", out.shape, out[0, :4])
